# revision 25
# baseline (speedup 1.0000x reference)
"""CIN (Compressed Interaction Network) kernel for Trainium2, 8 NeuronCores.

Problem: x [4096, 39, 16]; 3 CIN layers (size 128 each):
  out_k[b,s,d] = sum_{i,j} x[b,i,d] * prev[b,j,d] * w_k[i*Fk+j, s] + b_k[s]
Output: sum_d concat(out_1, out_2) -> [4096, 256]  (layer 0 output dropped)

Strategy (data-parallel, batch sharded 8 ways, 512 rows/core):
  Layer 0: host packs z0 = x_i*x_j for symmetric pairs (i<=j, w0 folded) as
  [819, bd] bf16; device contracts with w0 chunks on the PE (fp32 PSUM).
  Layer 1: activations feature-on-partition (out0T [128, bd], bd=(b,d)
  b-major, 8192/core). z tiles [128, bd-half] built as (DMA
  partition-broadcast of x rows) * out0T via DVE tensor_tensor (bf16),
  contracted with w1 chunks on the PE accumulating over i.
  Layer 2 never computes out2[b,s,d]: since only sum_d out2 is needed,
    sum_d out2[b,:,d] = w2^T . vec(G2_b) + 16*b2,
    G2_b[i,j] = sum_d x[b,i,d]*out1[b,j,d]  (per-sample Gram, 14x fewer
  FLOPs). G2 via PE-transposing out1T 128-col blocks (8 b's each) times a
  host-built block-diagonal probe xBD; an extra ones-block in xBD yields
  outF1 = sum_d out1 from the same matmul. w2 then contracts G2.
"""
import sys

for p in ("/opt/trn_rl_repo",):
    if p not in sys.path:
        sys.path.insert(0, p)

import contextlib

import numpy as np
import ml_dtypes

import concourse.bass as bass
import concourse.mybir as mybir
import concourse.tile as tile
from concourse import bacc
from concourse.bass_utils import run_bass_kernel_spmd
from concourse.masks import make_identity

F32 = mybir.dt.float32
BF16 = mybir.dt.bfloat16

N_CORES = 8
B, F0, D = 4096, 39, 16
S = 128                      # each CIN layer size
BC = B // N_CORES            # 512 batch rows per core
BD = BC * D                  # 8192
QW = 4096                    # half width (8 PSUM banks of 512)
NQ = BD // QW                # 2
NG = QW // 512               # 8 groups of 512 per half
M1 = F0 * S                  # 4992
NP0 = F0 * (F0 + 1) // 2     # 780 symmetric pairs (i<=j)
C0 = 117                     # layer-0 m-chunk rows
NC0 = (NP0 + C0 - 1) // C0   # 7 chunks (last padded to 819)
M0P = NC0 * C0               # 819
_PAIRS = [(i, j) for i in range(F0) for j in range(i, F0)]
_PAIRS += [(0, 0)] * (M0P - NP0)

GB = 8                       # b's per Gram group (one 128-col transpose block)
NGRP = BC // GB              # 64 Gram groups
GN = GB * F0                 # 312 Gram data cols per group
GN2 = GN + GB                # 320: + ones-block producing sum_d out1
FB = 256                     # final matmul b-batch (N=256)


def build_program():
    nc = bacc.Bacc("TRN2", target_bir_lowering=False, debug=False,
                   num_devices=N_CORES)
    z0h = nc.dram_tensor("z0h", [M0P, BD], BF16, kind="ExternalInput").ap()
    xT = nc.dram_tensor("xT", [F0, BD], BF16, kind="ExternalInput").ap()
    xBD = nc.dram_tensor("xBD", [128, NGRP * GN2], BF16, kind="ExternalInput").ap()
    # weights host-packed to the exact SBUF layouts (one DMA each)
    w0 = nc.dram_tensor("w0", [C0, NC0 * S], BF16, kind="ExternalInput").ap()
    w1 = nc.dram_tensor("w1", [128, (M1 // 128) * S], BF16, kind="ExternalInput").ap()
    w2p = nc.dram_tensor("w2p", [128, (M1 // 128) * S], BF16, kind="ExternalInput").ap()
    b0 = nc.dram_tensor("b0", [S, 1], F32, kind="ExternalInput").ap()
    b1 = nc.dram_tensor("b1", [S, 1], F32, kind="ExternalInput").ap()
    b2x = nc.dram_tensor("b2x", [S, 1], F32, kind="ExternalInput").ap()  # 16*b2
    out = nc.dram_tensor("out", [BC, 2 * S], F32, kind="ExternalOutput").ap()

    with tile.TileContext(nc) as tc:
        _body(nc, tc, z0h, xT, xBD, w0, w1, w2p, b0, b1, b2x, out)
    nc.compile()
    return nc


def _body(nc, tc, z0h, xT, xBD, w0, w1, w2p, b0, b1, b2x, out):
    ctx = contextlib.ExitStack()
    with ctx:
        const = ctx.enter_context(tc.tile_pool(name="const", bufs=1))
        acts = ctx.enter_context(tc.tile_pool(name="acts", bufs=1))
        wpool = ctx.enter_context(tc.tile_pool(name="w", bufs=1))
        bcast = ctx.enter_context(tc.tile_pool(name="bcast", bufs=3))
        zpool = ctx.enter_context(tc.tile_pool(name="z", bufs=4))
        o1tp = ctx.enter_context(tc.tile_pool(name="o1t", bufs=4))

        # ---- constants / weights ----
        b0t = const.tile([S, 1], F32, tag="b0")
        b1t = const.tile([S, 1], F32, tag="b1")
        b2xt = const.tile([S, 1], F32, tag="b2x")
        nc.scalar.dma_start(out=b0t[:], in_=b0[:])
        nc.scalar.dma_start(out=b1t[:], in_=b1[:])
        nc.scalar.dma_start(out=b2xt[:], in_=b2x[:])

        identb = const.tile([128, 128], BF16, tag="identb")
        identf = const.tile([128, 128], F32, tag="identf")
        make_identity(nc, identb[:])
        make_identity(nc, identf[:])

        w0sb = wpool.tile([C0, NC0 * S], BF16, tag="w0")
        nc.scalar.dma_start(out=w0sb[:], in_=w0[:])
        w1sb = wpool.tile([128, (M1 // 128) * S], BF16, tag="w1")
        nc.scalar.dma_start(out=w1sb[:], in_=w1[:])
        w2sb = wpool.tile([128, (M1 // 128) * S], BF16, tag="w2sb")
        nc.scalar.dma_start(out=w2sb[:], in_=w2p[:])
        xbdt = wpool.tile([128, NGRP * GN2], BF16, tag="xbd")
        nc.scalar.dma_start(out=xbdt[:], in_=xBD[:])

        out0T = acts.tile([S, BD], BF16, tag="out0T")
        out1T = acts.tile([S, BD], BF16, tag="out1T")
        outF1 = acts.tile([S, BC], F32, tag="outF1")
        outF2 = acts.tile([S, BC], F32, tag="outF2")
        g2s = acts.tile([S, F0 * BC], BF16, tag="g2s")  # col = BC*i + b

        IDENT_ACT = mybir.ActivationFunctionType.Identity

        # Four 2048-col quarters, software-pipelined: L0/L1 of quarter k use
        # PSUM banks 0-3 (ps pool); Gram/final/assembly tiles live in the
        # other banks (pg pool) so quarter k's Gram work overlaps quarter
        # k+1's L0/L1, filling the DVE idle holes and hiding the tail.
        NQT = 4
        QT = BD // NQT           # 2048
        NGQ = QT // 512          # 4 psum bank-groups per quarter
        GPQ = QT // 128          # 16 Gram groups per quarter

        psum = ctx.enter_context(tc.tile_pool(name="ps", bufs=1, space="PSUM"))
        pgram = ctx.enter_context(tc.tile_pool(name="pg", bufs=1, space="PSUM"))

        def emit_gram(grp):
            # G2_b[i,j] = sum_d x[b,i,d] * out1[b,j,d]; 8 b's per matmul.
            # out1T 128-col block transposed via HWDGE DMA-transpose
            # (SBUF->SBUF, bf16) -- keeps the PE and ACT out of the chain.
            o1t = o1tp.tile([128, 128], BF16, tag="o1t")
            nc.scalar.dma_start(out=o1t[:],
                                in_=out1T[:, 128 * grp:128 * (grp + 1)],
                                transpose=True)
            g2p = pgram.tile([128, GN2], F32, tag="g2p", name="g2p")
            nc.tensor.matmul(g2p[:], o1t[:], xbdt[:, GN2 * grp:GN2 * (grp + 1)],
                             start=True, stop=True)
            # drain [jj, (i, k)] -> g2s cols BC*i + GB*grp + k (i-major)
            dst = (g2s[:].rearrange("p (i b) -> p i b", i=F0)
                   [:, :, GB * grp:GB * (grp + 1)])
            nc.vector.tensor_copy(dst, g2p[:, 0:GN].rearrange("p (i k) -> p i k",
                                                              i=F0))
            # ones-block: outF1[s, b] = sum_d out1 (b1 already folded in out1T)
            nc.scalar.activation(outF1[:, GB * grp:GB * (grp + 1)],
                                 g2p[:, GN:GN2],
                                 mybir.ActivationFunctionType.Copy)

        def emit_final(sgi):
            # outF2[:, b] = sum_m2 w2[m2,:]^T G2[b, m2] + 16*b2
            facc = pgram.tile([S, FB], F32, tag="facc", name="facc")
            for i in range(F0):
                nc.tensor.matmul(facc[:], w2sb[:, S * i:S * (i + 1)],
                                 g2s[:, BC * i + FB * sgi:BC * i + FB * (sgi + 1)],
                                 start=(i == 0), stop=(i == F0 - 1))
            nc.scalar.activation(outF2[:, FB * sgi:FB * (sgi + 1)], facc[:],
                                 IDENT_ACT, bias=b2xt[:], scale=1.0)

        def emit_assembly(t):
            csl = slice(128 * t, 128 * (t + 1))
            otile = o1tp.tile([128, 2 * S], F32, tag="outsb")
            p1 = pgram.tile([128, 128], F32, tag="tr", name="tr")
            nc.tensor.transpose(p1[:], outF1[:, csl], identf[:])
            nc.vector.tensor_copy(otile[:, 0:S], p1[:])
            p2 = pgram.tile([128, 128], F32, tag="tr", name="tr")
            nc.tensor.transpose(p2[:], outF2[:, csl], identf[:])
            nc.vector.tensor_copy(otile[:, S:2 * S], p2[:])
            nc.sync.dma_start(out=out[csl, :], in_=otile[:])

        for k in range(NQT):
            qsl = slice(k * QT, (k + 1) * QT)
            # ---- layer 0, quarter k ----
            accs = [psum.tile([S, 512], F32, tag=f"acc{g}", name=f"acc{g}")
                    for g in range(NGQ)]
            for c in range(0, NC0, 2):
                nchunk = min(2, NC0 - c)
                z0t = zpool.tile([128, 2 * QT], BF16, tag="z")
                nc.sync.dma_start(
                    out=z0t[0:C0, :].rearrange("p (two w) -> p two w", two=2)
                    [:, 0:nchunk, :],
                    in_=z0h[C0 * c:C0 * (c + nchunk), qsl].rearrange(
                        "(two p) w -> p two w", two=nchunk))
                for sub in range(nchunk):
                    lhsT = w0sb[:, S * (c + sub):S * (c + sub + 1)]
                    for g in range(NGQ):
                        nc.tensor.matmul(
                            accs[g][:], lhsT,
                            z0t[0:C0, sub * QT + 512 * g:sub * QT + 512 * (g + 1)],
                            start=(c + sub == 0), stop=(c + sub == NC0 - 1))
            for g in range(NGQ):
                nc.scalar.activation(out0T[:, k * QT + 512 * g:k * QT + 512 * (g + 1)],
                                     accs[g][:], IDENT_ACT, bias=b0t[:], scale=1.0)

            # ---- layer 1, quarter k ----
            accs2 = [psum.tile([S, 512], F32, tag=f"acc{g}", name=f"acc{g}")
                     for g in range(NGQ)]
            # i's processed in pairs: one broadcast DMA + one DVE multiply
            # covers two i rows (halves per-op overhead and issue count)
            for ib in range((F0 + 1) // 2):
                i0 = 2 * ib
                ni = min(2, F0 - i0)
                bc_t = bcast.tile([128, 2 * QT], BF16, tag="bc")
                src = (xT[i0:i0 + ni, qsl].unsqueeze(0)
                       .partition_broadcast(128))
                nc.sync.dma_start(out=bc_t[:].rearrange(
                    "p (two w) -> p two w", two=2)[:, 0:ni, :], in_=src)
                z = zpool.tile([128, 2 * QT], BF16, tag="z")
                in0 = (out0T[:, qsl].unsqueeze(1)
                       .broadcast_to([128, ni, QT]))
                nc.vector.tensor_tensor(
                    out=z[:].rearrange("p (two w) -> p two w", two=2)[:, 0:ni, :],
                    in0=in0,
                    in1=bc_t[:].rearrange("p (two w) -> p two w", two=2)[:, 0:ni, :],
                    op=mybir.AluOpType.mult)
                for sub in range(ni):
                    i = i0 + sub
                    lhsT = w1sb[:, S * i:S * (i + 1)]
                    for g in range(NGQ):
                        nc.tensor.matmul(accs2[g][:], lhsT,
                                         z[:, sub * QT + 512 * g:
                                           sub * QT + 512 * (g + 1)],
                                         start=(i == 0), stop=(i == F0 - 1))
            for g in range(NGQ):
                gco = k * QT + 512 * g
                nc.scalar.activation(out1T[:, gco:gco + 512], accs2[g][:],
                                     IDENT_ACT, bias=b1t[:], scale=1.0)

            # ---- layer 2 Gram for quarter k (overlaps quarter k+1) ----
            for grp in range(GPQ * k, GPQ * (k + 1)):
                emit_gram(grp)
            if k == 1:
                emit_final(0)
                emit_assembly(0)
                emit_assembly(1)
        emit_final(1)
        emit_assembly(2)
        emit_assembly(3)


_PROGRAM_CACHE = {}


def _get_program():
    if "nc" not in _PROGRAM_CACHE:
        _PROGRAM_CACHE["nc"] = build_program()
    return _PROGRAM_CACHE["nc"]


def host_prep(x, w0, b0, w1, b1, w2, b2):
    bf = ml_dtypes.bfloat16
    x = np.asarray(x, dtype=np.float32)
    II = np.array([p[0] for p in _PAIRS], np.int64)
    JJ = np.array([p[1] for p in _PAIRS], np.int64)
    w0f = np.asarray(w0, np.float32).reshape(F0, F0, S)
    w0s = w0f[II, JJ] + np.where((II != JJ)[:, None], w0f[JJ, II], 0.0)
    w0s[NP0:] = 0.0  # padded pair rows: weight 0 so duplicates don't count
    # pack to SBUF layouts: chunk-major column blocks, one DMA each
    w0b = np.ascontiguousarray(
        w0s.reshape(NC0, C0, S).transpose(1, 0, 2).reshape(C0, NC0 * S)
        .astype(bf))
    w1b = np.ascontiguousarray(
        np.asarray(w1, np.float32).reshape(F0, 128, S).transpose(1, 0, 2)
        .reshape(128, F0 * S).astype(bf))
    w2b = np.ascontiguousarray(
        np.asarray(w2, np.float32).reshape(F0, 128, S).transpose(1, 0, 2)
        .reshape(128, F0 * S).astype(bf))
    b0v = np.ascontiguousarray(np.asarray(b0, np.float32).reshape(S, 1))
    b1v = np.ascontiguousarray(np.asarray(b1, np.float32).reshape(S, 1))
    b2xv = np.ascontiguousarray(D * np.asarray(b2, np.float32).reshape(S, 1))

    in_maps = []
    for c in range(N_CORES):
        xs = x[BC * c:BC * (c + 1)]                       # [512, 39, 16]
        xTv = np.ascontiguousarray(
            xs.transpose(1, 0, 2).reshape(F0, BD).astype(bf))
        z0 = np.ascontiguousarray(
            (xs[:, II, :] * xs[:, JJ, :]).transpose(1, 0, 2)
            .reshape(M0P, BD).astype(bf))
        # block-diag probe: row 16k+d of group g -> x[8g+k, i, d] at col 8i+k;
        # cols 312..319: ones-block (col 312+k, rows 16k..16k+15) -> sum_d out1
        xbd = np.zeros((128, NGRP, GN2), np.float32)
        blk = xs.reshape(NGRP, GB, F0, D)                 # [g, k, i, d]
        for k in range(GB):
            xbd[16 * k:16 * (k + 1), :, k:GN:GB] = (
                blk[:, k].transpose(2, 0, 1))             # [d, g, i]
            xbd[16 * k:16 * (k + 1), :, GN + k] = 1.0
        xbd = np.ascontiguousarray(xbd.reshape(128, NGRP * GN2).astype(bf))
        in_maps.append({"z0h": z0, "xT": xTv, "xBD": xbd, "w0": w0b,
                        "w1": w1b, "w2p": w2b,
                        "b0": b0v, "b1": b1v, "b2x": b2xv})
    return in_maps


def kernel(x, w0, b0, w1, b1, w2, b2):
    in_maps = host_prep(x, w0, b0, w1, b1, w2, b2)
    nc = _get_program()
    res = run_bass_kernel_spmd(nc, in_maps, core_ids=list(range(N_CORES)),
                               trace=False)
    return np.concatenate([r["out"] for r in res.results], axis=0)


# revision 26
# speedup vs baseline: 1.5993x; 1.5993x over previous
"""CIN (Compressed Interaction Network) kernel for Trainium2, 8 NeuronCores.

Problem: x [4096, 39, 16]; 3 CIN layers (size 128 each):
  out_k[b,s,d] = sum_{i,j} x[b,i,d] * prev[b,j,d] * w_k[i*Fk+j, s] + b_k[s]
Output: sum_d concat(out_1, out_2) -> [4096, 256]  (layer 0 output dropped)

Strategy (data-parallel, batch sharded 8 ways, 512 rows/core):
  Layer 0: host packs z0 = x_i*x_j for symmetric pairs (i<=j, w0 folded) as
  [819, bd] bf16; device contracts with w0 chunks on the PE (fp32 PSUM).
  Layer 1: activations feature-on-partition (out0T [128, bd], bd=(b,d)
  b-major, 8192/core). z tiles [128, bd-half] built as (DMA
  partition-broadcast of x rows) * out0T via DVE tensor_tensor (bf16),
  contracted with w1 chunks on the PE accumulating over i.
  Layer 2 never computes out2[b,s,d]: since only sum_d out2 is needed,
    sum_d out2[b,:,d] = w2^T . vec(G2_b) + 16*b2,
    G2_b[i,j] = sum_d x[b,i,d]*out1[b,j,d]  (per-sample Gram, 14x fewer
  FLOPs). G2 via PE-transposing out1T 128-col blocks (8 b's each) times a
  host-built block-diagonal probe xBD; an extra ones-block in xBD yields
  outF1 = sum_d out1 from the same matmul. w2 then contracts G2.
"""
import sys

for p in ("/opt/trn_rl_repo",):
    if p not in sys.path:
        sys.path.insert(0, p)

import contextlib

import numpy as np
import ml_dtypes

import concourse.bass as bass
import concourse.mybir as mybir
import concourse.tile as tile
from concourse import bacc
from concourse.bass_utils import run_bass_kernel_spmd
from concourse.masks import make_identity

F32 = mybir.dt.float32
BF16 = mybir.dt.bfloat16

N_CORES = 8
B, F0, D = 4096, 39, 16
S = 128                      # each CIN layer size
BC = B // N_CORES            # 512 batch rows per core
BD = BC * D                  # 8192
QW = 4096                    # half width (8 PSUM banks of 512)
NQ = BD // QW                # 2
NG = QW // 512               # 8 groups of 512 per half
M1 = F0 * S                  # 4992
NP0 = F0 * (F0 + 1) // 2     # 780 symmetric pairs (i<=j)
C0 = 117                     # layer-0 m-chunk rows
NC0 = (NP0 + C0 - 1) // C0   # 7 chunks (last padded to 819)
M0P = NC0 * C0               # 819
_PAIRS = [(i, j) for i in range(F0) for j in range(i, F0)]
_PAIRS += [(0, 0)] * (M0P - NP0)

GB = 8                       # b's per Gram group (one 128-col transpose block)
NGRP = BC // GB              # 64 Gram groups
GN = GB * F0                 # 312 Gram data cols per group
GN2 = GN + GB                # 320: + ones-block producing sum_d out1
FB = 256                     # final matmul b-batch (N=256)


def build_program():
    nc = bacc.Bacc("TRN2", target_bir_lowering=False, debug=False,
                   num_devices=N_CORES)
    z0h = nc.dram_tensor("z0h", [M0P, BD], BF16, kind="ExternalInput").ap()
    xT = nc.dram_tensor("xT", [F0, BD], BF16, kind="ExternalInput").ap()
    xBD = nc.dram_tensor("xBD", [128, NGRP * GN2], BF16, kind="ExternalInput").ap()
    # weights host-packed to the exact SBUF layouts (one DMA each)
    w0 = nc.dram_tensor("w0", [C0, NC0 * S], BF16, kind="ExternalInput").ap()
    w1 = nc.dram_tensor("w1", [128, (M1 // 128) * S], BF16, kind="ExternalInput").ap()
    w2p = nc.dram_tensor("w2p", [128, (M1 // 128) * S], BF16, kind="ExternalInput").ap()
    b0 = nc.dram_tensor("b0", [S, 1], F32, kind="ExternalInput").ap()
    b1 = nc.dram_tensor("b1", [S, 1], F32, kind="ExternalInput").ap()
    b2x = nc.dram_tensor("b2x", [S, 1], F32, kind="ExternalInput").ap()  # 16*b2
    out = nc.dram_tensor("out", [BC, 2 * S], F32, kind="ExternalOutput").ap()

    with tile.TileContext(nc) as tc:
        _body(nc, tc, z0h, xT, xBD, w0, w1, w2p, b0, b1, b2x, out)
    nc.compile()
    return nc


def _body(nc, tc, z0h, xT, xBD, w0, w1, w2p, b0, b1, b2x, out):
    ctx = contextlib.ExitStack()
    with ctx:
        const = ctx.enter_context(tc.tile_pool(name="const", bufs=1))
        acts = ctx.enter_context(tc.tile_pool(name="acts", bufs=1))
        wpool = ctx.enter_context(tc.tile_pool(name="w", bufs=1))
        bcast = ctx.enter_context(tc.tile_pool(name="bcast", bufs=3))
        zpool = ctx.enter_context(tc.tile_pool(name="z", bufs=4))
        o1tp = ctx.enter_context(tc.tile_pool(name="o1t", bufs=4))

        # ---- constants / weights ----
        b0t = const.tile([S, 1], F32, tag="b0")
        b1t = const.tile([S, 1], F32, tag="b1")
        b2xt = const.tile([S, 1], F32, tag="b2x")
        nc.scalar.dma_start(out=b0t[:], in_=b0[:])
        nc.scalar.dma_start(out=b1t[:], in_=b1[:])
        nc.scalar.dma_start(out=b2xt[:], in_=b2x[:])

        identb = const.tile([128, 128], BF16, tag="identb")
        identf = const.tile([128, 128], F32, tag="identf")
        make_identity(nc, identb[:])
        make_identity(nc, identf[:])

        w0sb = wpool.tile([C0, NC0 * S], BF16, tag="w0")
        nc.scalar.dma_start(out=w0sb[:], in_=w0[:])
        w1sb = wpool.tile([128, (M1 // 128) * S], BF16, tag="w1")
        nc.scalar.dma_start(out=w1sb[:], in_=w1[:])
        w2sb = wpool.tile([128, (M1 // 128) * S], BF16, tag="w2sb")
        nc.scalar.dma_start(out=w2sb[:], in_=w2p[:])
        xbdt = wpool.tile([128, NGRP * GN2], BF16, tag="xbd")
        nc.scalar.dma_start(out=xbdt[:], in_=xBD[:])

        out0T = acts.tile([S, BD], BF16, tag="out0T")
        out1T = acts.tile([S, BD], BF16, tag="out1T")
        outF1 = acts.tile([S, BC], F32, tag="outF1")
        outF2 = acts.tile([S, BC], F32, tag="outF2")
        g2s = acts.tile([S, F0 * BC], BF16, tag="g2s")  # col = BC*i + b

        IDENT_ACT = mybir.ActivationFunctionType.Identity

        # Four 2048-col quarters, software-pipelined: L0/L1 of quarter k use
        # PSUM banks 0-3 (ps pool); Gram/final/assembly tiles live in the
        # other banks (pg pool) so quarter k's Gram work overlaps quarter
        # k+1's L0/L1, filling the DVE idle holes and hiding the tail.
        NQT = 4
        QT = BD // NQT           # 2048
        NGQ = QT // 512          # 4 psum bank-groups per quarter
        GPQ = QT // 128          # 16 Gram groups per quarter

        psum = ctx.enter_context(tc.tile_pool(name="ps", bufs=1, space="PSUM"))
        pgram = ctx.enter_context(tc.tile_pool(name="pg", bufs=1, space="PSUM"))

        def emit_gram(grp):
            # G2_b[i,j] = sum_d x[b,i,d] * out1[b,j,d]; 8 b's per matmul.
            tps = pgram.tile([128, 128], BF16, tag="tps", name="tps")
            nc.tensor.transpose(tps[:], out1T[:, 128 * grp:128 * (grp + 1)],
                                identb[:])
            o1t = o1tp.tile([128, 128], BF16, tag="o1t")
            nc.scalar.activation(o1t[:], tps[:],
                                 mybir.ActivationFunctionType.Copy)
            g2p = pgram.tile([128, GN2], F32, tag="g2p", name="g2p")
            nc.tensor.matmul(g2p[:], o1t[:], xbdt[:, GN2 * grp:GN2 * (grp + 1)],
                             start=True, stop=True)
            # drain [jj, (i, k)] -> g2s cols BC*i + GB*grp + k (i-major)
            dst = (g2s[:].rearrange("p (i b) -> p i b", i=F0)
                   [:, :, GB * grp:GB * (grp + 1)])
            nc.vector.tensor_copy(dst, g2p[:, 0:GN].rearrange("p (i k) -> p i k",
                                                              i=F0))
            # ones-block: outF1[s, b] = sum_d out1 (b1 already folded in out1T)
            nc.scalar.activation(outF1[:, GB * grp:GB * (grp + 1)],
                                 g2p[:, GN:GN2],
                                 mybir.ActivationFunctionType.Copy)

        def emit_final(sgi):
            # outF2[:, b] = sum_m2 w2[m2,:]^T G2[b, m2] + 16*b2
            facc = pgram.tile([S, FB], F32, tag="facc", name="facc")
            for i in range(F0):
                nc.tensor.matmul(facc[:], w2sb[:, S * i:S * (i + 1)],
                                 g2s[:, BC * i + FB * sgi:BC * i + FB * (sgi + 1)],
                                 start=(i == 0), stop=(i == F0 - 1))
            nc.scalar.activation(outF2[:, FB * sgi:FB * (sgi + 1)], facc[:],
                                 IDENT_ACT, bias=b2xt[:], scale=1.0)

        def emit_assembly(t):
            csl = slice(128 * t, 128 * (t + 1))
            otile = o1tp.tile([128, 2 * S], F32, tag="outsb")
            p1 = pgram.tile([128, 128], F32, tag="tr", name="tr")
            nc.tensor.transpose(p1[:], outF1[:, csl], identf[:])
            nc.vector.tensor_copy(otile[:, 0:S], p1[:])
            p2 = pgram.tile([128, 128], F32, tag="tr", name="tr")
            nc.tensor.transpose(p2[:], outF2[:, csl], identf[:])
            nc.vector.tensor_copy(otile[:, S:2 * S], p2[:])
            nc.sync.dma_start(out=out[csl, :], in_=otile[:])

        for k in range(NQT):
            qsl = slice(k * QT, (k + 1) * QT)
            # ---- layer 0, quarter k ----
            accs = [psum.tile([S, 512], F32, tag=f"acc{g}", name=f"acc{g}")
                    for g in range(NGQ)]
            for c in range(0, NC0, 2):
                nchunk = min(2, NC0 - c)
                z0t = zpool.tile([128, 2 * QT], BF16, tag="z")
                nc.sync.dma_start(
                    out=z0t[0:C0, :].rearrange("p (two w) -> p two w", two=2)
                    [:, 0:nchunk, :],
                    in_=z0h[C0 * c:C0 * (c + nchunk), qsl].rearrange(
                        "(two p) w -> p two w", two=nchunk))
                for sub in range(nchunk):
                    lhsT = w0sb[:, S * (c + sub):S * (c + sub + 1)]
                    for g in range(NGQ):
                        nc.tensor.matmul(
                            accs[g][:], lhsT,
                            z0t[0:C0, sub * QT + 512 * g:sub * QT + 512 * (g + 1)],
                            start=(c + sub == 0), stop=(c + sub == NC0 - 1))
            for g in range(NGQ):
                nc.scalar.activation(out0T[:, k * QT + 512 * g:k * QT + 512 * (g + 1)],
                                     accs[g][:], IDENT_ACT, bias=b0t[:], scale=1.0)

            # ---- layer 1, quarter k ----
            accs2 = [psum.tile([S, 512], F32, tag=f"acc{g}", name=f"acc{g}")
                     for g in range(NGQ)]
            # i's processed in pairs: one broadcast DMA + one DVE multiply
            # covers two i rows (halves per-op overhead and issue count)
            for ib in range((F0 + 1) // 2):
                i0 = 2 * ib
                ni = min(2, F0 - i0)
                bc_t = bcast.tile([128, 2 * QT], BF16, tag="bc")
                src = (xT[i0:i0 + ni, qsl].unsqueeze(0)
                       .partition_broadcast(128))
                nc.sync.dma_start(out=bc_t[:].rearrange(
                    "p (two w) -> p two w", two=2)[:, 0:ni, :], in_=src)
                z = zpool.tile([128, 2 * QT], BF16, tag="z")
                in0 = (out0T[:, qsl].unsqueeze(1)
                       .broadcast_to([128, ni, QT]))
                nc.vector.tensor_tensor(
                    out=z[:].rearrange("p (two w) -> p two w", two=2)[:, 0:ni, :],
                    in0=in0,
                    in1=bc_t[:].rearrange("p (two w) -> p two w", two=2)[:, 0:ni, :],
                    op=mybir.AluOpType.mult)
                for sub in range(ni):
                    i = i0 + sub
                    lhsT = w1sb[:, S * i:S * (i + 1)]
                    for g in range(NGQ):
                        nc.tensor.matmul(accs2[g][:], lhsT,
                                         z[:, sub * QT + 512 * g:
                                           sub * QT + 512 * (g + 1)],
                                         start=(i == 0), stop=(i == F0 - 1))
            for g in range(NGQ):
                gco = k * QT + 512 * g
                nc.scalar.activation(out1T[:, gco:gco + 512], accs2[g][:],
                                     IDENT_ACT, bias=b1t[:], scale=1.0)

            # ---- layer 2 Gram for quarter k (overlaps quarter k+1) ----
            for grp in range(GPQ * k, GPQ * (k + 1)):
                emit_gram(grp)
            if k == 1:
                emit_final(0)
                emit_assembly(0)
                emit_assembly(1)
        emit_final(1)
        emit_assembly(2)
        emit_assembly(3)


_PROGRAM_CACHE = {}


def _get_program():
    if "nc" not in _PROGRAM_CACHE:
        _PROGRAM_CACHE["nc"] = build_program()
    return _PROGRAM_CACHE["nc"]


def host_prep(x, w0, b0, w1, b1, w2, b2):
    bf = ml_dtypes.bfloat16
    x = np.asarray(x, dtype=np.float32)
    II = np.array([p[0] for p in _PAIRS], np.int64)
    JJ = np.array([p[1] for p in _PAIRS], np.int64)
    w0f = np.asarray(w0, np.float32).reshape(F0, F0, S)
    w0s = w0f[II, JJ] + np.where((II != JJ)[:, None], w0f[JJ, II], 0.0)
    w0s[NP0:] = 0.0  # padded pair rows: weight 0 so duplicates don't count
    # pack to SBUF layouts: chunk-major column blocks, one DMA each
    w0b = np.ascontiguousarray(
        w0s.reshape(NC0, C0, S).transpose(1, 0, 2).reshape(C0, NC0 * S)
        .astype(bf))
    w1b = np.ascontiguousarray(
        np.asarray(w1, np.float32).reshape(F0, 128, S).transpose(1, 0, 2)
        .reshape(128, F0 * S).astype(bf))
    w2b = np.ascontiguousarray(
        np.asarray(w2, np.float32).reshape(F0, 128, S).transpose(1, 0, 2)
        .reshape(128, F0 * S).astype(bf))
    b0v = np.ascontiguousarray(np.asarray(b0, np.float32).reshape(S, 1))
    b1v = np.ascontiguousarray(np.asarray(b1, np.float32).reshape(S, 1))
    b2xv = np.ascontiguousarray(D * np.asarray(b2, np.float32).reshape(S, 1))

    in_maps = []
    for c in range(N_CORES):
        xs = x[BC * c:BC * (c + 1)]                       # [512, 39, 16]
        xTv = np.ascontiguousarray(
            xs.transpose(1, 0, 2).reshape(F0, BD).astype(bf))
        z0 = np.ascontiguousarray(
            (xs[:, II, :] * xs[:, JJ, :]).transpose(1, 0, 2)
            .reshape(M0P, BD).astype(bf))
        # block-diag probe: row 16k+d of group g -> x[8g+k, i, d] at col 8i+k;
        # cols 312..319: ones-block (col 312+k, rows 16k..16k+15) -> sum_d out1
        xbd = np.zeros((128, NGRP, GN2), np.float32)
        blk = xs.reshape(NGRP, GB, F0, D)                 # [g, k, i, d]
        for k in range(GB):
            xbd[16 * k:16 * (k + 1), :, k:GN:GB] = (
                blk[:, k].transpose(2, 0, 1))             # [d, g, i]
            xbd[16 * k:16 * (k + 1), :, GN + k] = 1.0
        xbd = np.ascontiguousarray(xbd.reshape(128, NGRP * GN2).astype(bf))
        in_maps.append({"z0h": z0, "xT": xTv, "xBD": xbd, "w0": w0b,
                        "w1": w1b, "w2p": w2b,
                        "b0": b0v, "b1": b1v, "b2x": b2xv})
    return in_maps


def kernel(x, w0, b0, w1, b1, w2, b2):
    in_maps = host_prep(x, w0, b0, w1, b1, w2, b2)
    nc = _get_program()
    res = run_bass_kernel_spmd(nc, in_maps, core_ids=list(range(N_CORES)),
                               trace=False)
    return np.concatenate([r["out"] for r in res.results], axis=0)


# revision 27
# speedup vs baseline: 1.7703x; 1.1069x over previous
"""CIN (Compressed Interaction Network) kernel for Trainium2, 8 NeuronCores.

Problem: x [4096, 39, 16]; 3 CIN layers (size 128 each):
  out_k[b,s,d] = sum_{i,j} x[b,i,d] * prev[b,j,d] * w_k[i*Fk+j, s] + b_k[s]
Output: sum_d concat(out_1, out_2) -> [4096, 256]  (layer 0 output dropped)

Strategy (data-parallel, batch sharded 8 ways, 512 rows/core):
  Layer 0: host packs z0 = x_i*x_j for symmetric pairs (i<=j, w0 folded) as
  [819, bd] bf16; device contracts with w0 chunks on the PE (fp32 PSUM).
  Layer 1: activations feature-on-partition (out0T [128, bd], bd=(b,d)
  b-major, 8192/core). z tiles [128, bd-half] built as (DMA
  partition-broadcast of x rows) * out0T via DVE tensor_tensor (bf16),
  contracted with w1 chunks on the PE accumulating over i.
  Layer 2 never computes out2[b,s,d]: since only sum_d out2 is needed,
    sum_d out2[b,:,d] = w2^T . vec(G2_b) + 16*b2,
    G2_b[i,j] = sum_d x[b,i,d]*out1[b,j,d]  (per-sample Gram, 14x fewer
  FLOPs). G2 via PE-transposing out1T 128-col blocks (8 b's each) times a
  host-built block-diagonal probe xBD; an extra ones-block in xBD yields
  outF1 = sum_d out1 from the same matmul. w2 then contracts G2.
"""
import sys

for p in ("/opt/trn_rl_repo",):
    if p not in sys.path:
        sys.path.insert(0, p)

import contextlib

import numpy as np
import ml_dtypes

import concourse.bass as bass
import concourse.mybir as mybir
import concourse.tile as tile
from concourse import bacc
from concourse.bass_utils import run_bass_kernel_spmd
from concourse.masks import make_identity

F32 = mybir.dt.float32
BF16 = mybir.dt.bfloat16

N_CORES = 8
B, F0, D = 4096, 39, 16
S = 128                      # each CIN layer size
BC = B // N_CORES            # 512 batch rows per core
BD = BC * D                  # 8192
QW = 4096                    # half width (8 PSUM banks of 512)
NQ = BD // QW                # 2
NG = QW // 512               # 8 groups of 512 per half
M1 = F0 * S                  # 4992
NP0 = F0 * (F0 + 1) // 2     # 780 symmetric pairs (i<=j)
C0 = 117                     # layer-0 m-chunk rows
NC0 = (NP0 + C0 - 1) // C0   # 7 chunks (last padded to 819)
M0P = NC0 * C0               # 819
_PAIRS = [(i, j) for i in range(F0) for j in range(i, F0)]
_PAIRS += [(0, 0)] * (M0P - NP0)

GB = 8                       # b's per Gram group (one 128-col transpose block)
NGRP = BC // GB              # 64 Gram groups
GN = GB * F0                 # 312 Gram data cols per group
GN2 = GN + GB                # 320: + ones-block producing sum_d out1
FB = 256                     # final matmul b-batch (N=256)


def build_program():
    nc = bacc.Bacc("TRN2", target_bir_lowering=False, debug=False,
                   num_devices=N_CORES)
    z0h = nc.dram_tensor("z0h", [M0P, BD], BF16, kind="ExternalInput").ap()
    xT = nc.dram_tensor("xT", [F0, BD], BF16, kind="ExternalInput").ap()
    xBD = nc.dram_tensor("xBD", [128, NGRP * GN2], BF16, kind="ExternalInput").ap()
    # weights host-packed to the exact SBUF layouts (one DMA each)
    w0 = nc.dram_tensor("w0", [C0, NC0 * S], BF16, kind="ExternalInput").ap()
    w1 = nc.dram_tensor("w1", [128, (M1 // 128) * S], BF16, kind="ExternalInput").ap()
    w2p = nc.dram_tensor("w2p", [128, (M1 // 128) * S], BF16, kind="ExternalInput").ap()
    b0 = nc.dram_tensor("b0", [S, 1], F32, kind="ExternalInput").ap()
    b1 = nc.dram_tensor("b1", [S, 1], F32, kind="ExternalInput").ap()
    b2x = nc.dram_tensor("b2x", [S, 1], F32, kind="ExternalInput").ap()  # 16*b2
    out = nc.dram_tensor("out", [BC, 2 * S], F32, kind="ExternalOutput").ap()

    with tile.TileContext(nc) as tc:
        _body(nc, tc, z0h, xT, xBD, w0, w1, w2p, b0, b1, b2x, out)
    nc.compile()
    return nc


def _body(nc, tc, z0h, xT, xBD, w0, w1, w2p, b0, b1, b2x, out):
    ctx = contextlib.ExitStack()
    with ctx:
        const = ctx.enter_context(tc.tile_pool(name="const", bufs=1))
        acts = ctx.enter_context(tc.tile_pool(name="acts", bufs=1))
        wpool = ctx.enter_context(tc.tile_pool(name="w", bufs=1))
        bcast = ctx.enter_context(tc.tile_pool(name="bcast", bufs=6))
        zpool = ctx.enter_context(tc.tile_pool(name="z", bufs=4))
        o1tp = ctx.enter_context(tc.tile_pool(name="o1t", bufs=4))

        # ---- constants / weights ----
        b0t = const.tile([S, 1], F32, tag="b0")
        b1t = const.tile([S, 1], F32, tag="b1")
        b2xt = const.tile([S, 1], F32, tag="b2x")
        nc.scalar.dma_start(out=b0t[:], in_=b0[:])
        nc.scalar.dma_start(out=b1t[:], in_=b1[:])
        nc.scalar.dma_start(out=b2xt[:], in_=b2x[:])

        identb = const.tile([128, 128], BF16, tag="identb")
        identf = const.tile([128, 128], F32, tag="identf")
        make_identity(nc, identb[:])
        make_identity(nc, identf[:])

        w0sb = wpool.tile([C0, NC0 * S], BF16, tag="w0")
        nc.scalar.dma_start(out=w0sb[:], in_=w0[:])
        w1sb = wpool.tile([128, (M1 // 128) * S], BF16, tag="w1")
        nc.scalar.dma_start(out=w1sb[:], in_=w1[:])
        w2sb = wpool.tile([128, (M1 // 128) * S], BF16, tag="w2sb")
        nc.scalar.dma_start(out=w2sb[:], in_=w2p[:])
        xbdt = wpool.tile([128, NGRP * GN2], BF16, tag="xbd")
        nc.scalar.dma_start(out=xbdt[:], in_=xBD[:])

        out0T = acts.tile([S, BD], BF16, tag="out0T")
        out1T = acts.tile([S, BD], BF16, tag="out1T")
        outF1 = acts.tile([S, BC], F32, tag="outF1")
        outF2 = acts.tile([S, BC], F32, tag="outF2")
        g2s = acts.tile([S, F0 * BC], BF16, tag="g2s")  # col = BC*i + b

        IDENT_ACT = mybir.ActivationFunctionType.Identity

        # Four 2048-col quarters, software-pipelined: L0/L1 of quarter k use
        # PSUM banks 0-3 (ps pool); Gram/final/assembly tiles live in the
        # other banks (pg pool) so quarter k's Gram work overlaps quarter
        # k+1's L0/L1, filling the DVE idle holes and hiding the tail.
        NQT = 4
        QT = BD // NQT           # 2048
        NGQ = QT // 512          # 4 psum bank-groups per quarter
        GPQ = QT // 128          # 16 Gram groups per quarter

        psum = ctx.enter_context(tc.tile_pool(name="ps", bufs=1, space="PSUM"))
        pgram = ctx.enter_context(tc.tile_pool(name="pg", bufs=1, space="PSUM"))

        def emit_gram(grp):
            # G2_b[i,j] = sum_d x[b,i,d] * out1[b,j,d]; 8 b's per matmul.
            tps = pgram.tile([128, 128], BF16, tag="tps", name="tps")
            nc.tensor.transpose(tps[:], out1T[:, 128 * grp:128 * (grp + 1)],
                                identb[:])
            o1t = o1tp.tile([128, 128], BF16, tag="o1t")
            nc.scalar.activation(o1t[:], tps[:],
                                 mybir.ActivationFunctionType.Copy)
            g2p = pgram.tile([128, GN2], F32, tag="g2p", name="g2p")
            nc.tensor.matmul(g2p[:], o1t[:], xbdt[:, GN2 * grp:GN2 * (grp + 1)],
                             start=True, stop=True)
            # drain [jj, (i, k)] -> g2s cols BC*i + GB*grp + k (i-major)
            dst = (g2s[:].rearrange("p (i b) -> p i b", i=F0)
                   [:, :, GB * grp:GB * (grp + 1)])
            nc.vector.tensor_copy(dst, g2p[:, 0:GN].rearrange("p (i k) -> p i k",
                                                              i=F0))
            # ones-block: outF1[s, b] = sum_d out1 (b1 already folded in out1T)
            nc.scalar.activation(outF1[:, GB * grp:GB * (grp + 1)],
                                 g2p[:, GN:GN2],
                                 mybir.ActivationFunctionType.Copy)

        def emit_final(sgi):
            # outF2[:, b] = sum_m2 w2[m2,:]^T G2[b, m2] + 16*b2
            facc = pgram.tile([S, FB], F32, tag="facc", name="facc")
            for i in range(F0):
                nc.tensor.matmul(facc[:], w2sb[:, S * i:S * (i + 1)],
                                 g2s[:, BC * i + FB * sgi:BC * i + FB * (sgi + 1)],
                                 start=(i == 0), stop=(i == F0 - 1))
            nc.scalar.activation(outF2[:, FB * sgi:FB * (sgi + 1)], facc[:],
                                 IDENT_ACT, bias=b2xt[:], scale=1.0)

        def emit_assembly(t):
            csl = slice(128 * t, 128 * (t + 1))
            otile = o1tp.tile([128, 2 * S], F32, tag="outsb")
            p1 = pgram.tile([128, 128], F32, tag="tr", name="tr")
            nc.tensor.transpose(p1[:], outF1[:, csl], identf[:])
            nc.vector.tensor_copy(otile[:, 0:S], p1[:])
            p2 = pgram.tile([128, 128], F32, tag="tr", name="tr")
            nc.tensor.transpose(p2[:], outF2[:, csl], identf[:])
            nc.vector.tensor_copy(otile[:, S:2 * S], p2[:])
            nc.sync.dma_start(out=out[csl, :], in_=otile[:])

        for k in range(NQT):
            qsl = slice(k * QT, (k + 1) * QT)
            # ---- layer 0, quarter k ----
            accs = [psum.tile([S, 512], F32, tag=f"acc{g}", name=f"acc{g}")
                    for g in range(NGQ)]
            for c in range(NC0):
                z0t = zpool.tile([128, QT], BF16, tag="z")
                nc.sync.dma_start(out=z0t[0:C0, :],
                                  in_=z0h[C0 * c:C0 * (c + 1), qsl])
                lhsT = w0sb[:, S * c:S * (c + 1)]
                for g in range(NGQ):
                    nc.tensor.matmul(accs[g][:], lhsT,
                                     z0t[0:C0, 512 * g:512 * (g + 1)],
                                     start=(c == 0), stop=(c == NC0 - 1))
            for g in range(NGQ):
                nc.scalar.activation(out0T[:, k * QT + 512 * g:k * QT + 512 * (g + 1)],
                                     accs[g][:], IDENT_ACT, bias=b0t[:], scale=1.0)

            # ---- layer 1, quarter k ----
            accs2 = [psum.tile([S, 512], F32, tag=f"acc{g}", name=f"acc{g}")
                     for g in range(NGQ)]
            for i in range(F0):
                bc_t = bcast.tile([128, QT], BF16, tag="bc")
                nc.sync.dma_start(out=bc_t[:],
                                  in_=xT[i:i + 1, qsl].partition_broadcast(128))
                z = zpool.tile([128, QT], BF16, tag="z")
                nc.vector.tensor_tensor(out=z[:], in0=out0T[:, qsl], in1=bc_t[:],
                                        op=mybir.AluOpType.mult)
                lhsT = w1sb[:, S * i:S * (i + 1)]
                for g in range(NGQ):
                    nc.tensor.matmul(accs2[g][:], lhsT, z[:, 512 * g:512 * (g + 1)],
                                     start=(i == 0), stop=(i == F0 - 1))
            for g in range(NGQ):
                gco = k * QT + 512 * g
                nc.scalar.activation(out1T[:, gco:gco + 512], accs2[g][:],
                                     IDENT_ACT, bias=b1t[:], scale=1.0)

            # ---- layer 2 Gram for quarter k (overlaps quarter k+1) ----
            for grp in range(GPQ * k, GPQ * (k + 1)):
                emit_gram(grp)
            if k == 1:
                emit_final(0)
                emit_assembly(0)
                emit_assembly(1)
        emit_final(1)
        emit_assembly(2)
        emit_assembly(3)


_PROGRAM_CACHE = {}


def _get_program():
    if "nc" not in _PROGRAM_CACHE:
        _PROGRAM_CACHE["nc"] = build_program()
    return _PROGRAM_CACHE["nc"]


def host_prep(x, w0, b0, w1, b1, w2, b2):
    bf = ml_dtypes.bfloat16
    x = np.asarray(x, dtype=np.float32)
    II = np.array([p[0] for p in _PAIRS], np.int64)
    JJ = np.array([p[1] for p in _PAIRS], np.int64)
    w0f = np.asarray(w0, np.float32).reshape(F0, F0, S)
    w0s = w0f[II, JJ] + np.where((II != JJ)[:, None], w0f[JJ, II], 0.0)
    w0s[NP0:] = 0.0  # padded pair rows: weight 0 so duplicates don't count
    # pack to SBUF layouts: chunk-major column blocks, one DMA each
    w0b = np.ascontiguousarray(
        w0s.reshape(NC0, C0, S).transpose(1, 0, 2).reshape(C0, NC0 * S)
        .astype(bf))
    w1b = np.ascontiguousarray(
        np.asarray(w1, np.float32).reshape(F0, 128, S).transpose(1, 0, 2)
        .reshape(128, F0 * S).astype(bf))
    w2b = np.ascontiguousarray(
        np.asarray(w2, np.float32).reshape(F0, 128, S).transpose(1, 0, 2)
        .reshape(128, F0 * S).astype(bf))
    b0v = np.ascontiguousarray(np.asarray(b0, np.float32).reshape(S, 1))
    b1v = np.ascontiguousarray(np.asarray(b1, np.float32).reshape(S, 1))
    b2xv = np.ascontiguousarray(D * np.asarray(b2, np.float32).reshape(S, 1))

    in_maps = []
    for c in range(N_CORES):
        xs = x[BC * c:BC * (c + 1)]                       # [512, 39, 16]
        xTv = np.ascontiguousarray(
            xs.transpose(1, 0, 2).reshape(F0, BD).astype(bf))
        z0 = np.ascontiguousarray(
            (xs[:, II, :] * xs[:, JJ, :]).transpose(1, 0, 2)
            .reshape(M0P, BD).astype(bf))
        # block-diag probe: row 16k+d of group g -> x[8g+k, i, d] at col 8i+k;
        # cols 312..319: ones-block (col 312+k, rows 16k..16k+15) -> sum_d out1
        xbd = np.zeros((128, NGRP, GN2), np.float32)
        blk = xs.reshape(NGRP, GB, F0, D)                 # [g, k, i, d]
        for k in range(GB):
            xbd[16 * k:16 * (k + 1), :, k:GN:GB] = (
                blk[:, k].transpose(2, 0, 1))             # [d, g, i]
            xbd[16 * k:16 * (k + 1), :, GN + k] = 1.0
        xbd = np.ascontiguousarray(xbd.reshape(128, NGRP * GN2).astype(bf))
        in_maps.append({"z0h": z0, "xT": xTv, "xBD": xbd, "w0": w0b,
                        "w1": w1b, "w2p": w2b,
                        "b0": b0v, "b1": b1v, "b2x": b2xv})
    return in_maps


def kernel(x, w0, b0, w1, b1, w2, b2):
    in_maps = host_prep(x, w0, b0, w1, b1, w2, b2)
    nc = _get_program()
    res = run_bass_kernel_spmd(nc, in_maps, core_ids=list(range(N_CORES)),
                               trace=False)
    return np.concatenate([r["out"] for r in res.results], axis=0)
